# revision 25
# baseline (speedup 1.0000x reference)
"""Block-sparse (view-causal) multi-head attention on 8 TRN2 NeuronCores.

Full inputs in, full output out. Sharding: data-parallel over batch (B=2),
tensor-parallel over heads (16 heads -> 4 per core). Each core computes its
4 heads' attention + its slice of the output projection; the host sums the
4 head-group partial projections per batch (the tensor-parallel reduce).

v2c:
- flipped PV: outT = va.T @ et with va stationary (N=256 moving free dim),
  the two heads of a pair packed into PE column groups; attention output
  lands channel-major so no PE transposes are needed before the projection.
- softmax denominators from ones-weight matmuls into the spare column
  groups, reduced+broadcast across partitions by one block-diagonal-map
  matmul, inverted with the fast DVE reciprocal.
- everything stays bf16 (fp8 measured ~5e-2 rel err: quantization noise
  in a random dot product does not average down vs the signal). Inputs
  arrive as a few large host-packed DMAs; attention scale folded into Wq.
- paced interleaved emission: the exp stream on the Scalar engine starts
  right after the first q/k projection slice; projections, PV, dR and the
  output projection drain into the gaps between score blocks. Pair 1 is
  processed in descending q-view order so the tail ends on the smallest
  views.
"""

import sys

if "/opt/trn_rl_repo" not in sys.path:
    sys.path.insert(0, "/opt/trn_rl_repo")

import numpy as np
import ml_dtypes

B, V, L, C, H = 2, 8, 256, 1024, 16
S = V * L                # 2048 tokens
DH = C // H              # 64
HPC = 4                  # heads per core
CPB = HPC * DH           # 256 channel block per core
N_CORES = 8
SCALE = DH ** -0.5       # 1/8, folded into Wq host-side
_compiled = {}
LAST_RESULTS = None


def _allowed(qv):
    """View-level mask row: views 0/1 cross-attend only; views >=2 block-causal."""
    if qv == 0:
        return [1]
    if qv == 1:
        return [0]
    return list(range(qv + 1))


def build():
    import concourse.tile as tile
    from concourse import bacc, mybir

    f32 = mybir.dt.float32
    bf16 = mybir.dt.bfloat16
    EXP = mybir.ActivationFunctionType.Exp
    MULT = mybir.AluOpType.mult

    nc = bacc.Bacc("TRN2", target_bir_lowering=False, debug=False,
                   num_devices=N_CORES)
    # host-packed layouts (row = 128*k + p for chunk k):
    #   xi [128, k(8), S] flat;  wq/wk/wv [128, k(8), CPB];  wp [128, k(2), C]
    xi_d = nc.dram_tensor("xi", [128, 8 * S], bf16, kind="ExternalInput").ap()
    wq_d = nc.dram_tensor("wq", [128, 8 * CPB], bf16,
                          kind="ExternalInput").ap()
    wk_d = nc.dram_tensor("wk", [128, 8 * CPB], bf16,
                          kind="ExternalInput").ap()
    wv_d = nc.dram_tensor("wv", [128, 8 * CPB], bf16,
                          kind="ExternalInput").ap()
    wp_d = nc.dram_tensor("wp", [128, 2 * C], bf16, kind="ExternalInput").ap()
    y = nc.dram_tensor("y", [S, C], bf16, kind="ExternalOutput").ap()

    KC = 8               # contraction chunks (of 128) for q/k/v projections
    NS = S // 512        # 4 free-dim chunks for q/k projections
    SC = S // 128        # 16 sequence chunks

    with tile.TileContext(nc) as tc:
        with (
            tc.tile_pool(name="xt", bufs=1) as xt_pool,
            tc.tile_pool(name="wts", bufs=1) as w_pool,
            tc.tile_pool(name="qk", bufs=1) as qk_pool,
            tc.tile_pool(name="va", bufs=SC) as va_pool,
            tc.tile_pool(name="ot", bufs=1) as ot_pool,
            tc.tile_pool(name="exp", bufs=3) as exp_pool,
            tc.tile_pool(name="small", bufs=1) as small_pool,
            tc.tile_pool(name="nrm", bufs=2) as nrm_pool,
            tc.tile_pool(name="ysb", bufs=3) as ysb_pool,
            tc.tile_pool(name="pse", bufs=2, space="PSUM") as psum_e,
            tc.tile_pool(name="psu", bufs=2, space="PSUM") as psum_u,
            tc.tile_pool(name="psp", bufs=2, space="PSUM") as psum_p,
        ):
            # ---- input DMAs (few, large) ----
            wq_t = w_pool.tile([128, 8 * CPB], bf16, tag="wq", name="wq")
            wk_t = w_pool.tile([128, 8 * CPB], bf16, tag="wk", name="wk")
            wv_t = w_pool.tile([128, 8 * CPB], bf16, tag="wv", name="wv")
            wp_t = w_pool.tile([128, 2 * C], bf16, tag="wp", name="wp")
            # weights first on the two fast HWDGE queues; x quarters behind
            # them; wp (needed last) on the slow gpsimd SWDGE path
            nc.sync.dma_start(wq_t[:], wq_d[:, :])
            nc.scalar.dma_start(wk_t[:], wk_d[:, :])
            xi_t = xt_pool.tile([128, 8 * S], bf16, tag="xi", name="xi")
            Q = 2 * S
            for i in range(2):
                nc.sync.dma_start(xi_t[:, i * Q:(i + 1) * Q],
                                  xi_d[:, i * Q:(i + 1) * Q])
                nc.scalar.dma_start(xi_t[:, (i + 2) * Q:(i + 3) * Q],
                                    xi_d[:, (i + 2) * Q:(i + 3) * Q])
            nc.scalar.dma_start(wv_t[:], wv_d[:, :])
            nc.gpsimd.dma_start(wp_t[:], wp_d[:, :])

            def xc(k):
                return xi_t[:, k * S:(k + 1) * S]

            def wc(wt, k):
                return wt[:, k * CPB:(k + 1) * CPB]

            # ---- constants ----
            ones32 = small_pool.tile([128, 32], bf16, tag="ones32")
            nc.vector.memset(ones32[:], 1.0 / 32.0)
            zeros128 = small_pool.tile([128, 128], bf16, tag="zeros128")
            nc.vector.memset(zeros128[:], 0.0)
            # block-diagonal ones map: one matmul does the 64-row partition
            # reduce AND broadcasts d_h0 to rows 0-63 / d_h1 to rows 64-127
            map128 = small_pool.tile([128, 128], f32, tag="map128")
            nc.vector.memset(map128[:], 0.0)
            nc.vector.memset(map128[0:64, 0:64], 1.0)
            nc.vector.memset(map128[64:128, 64:128], 1.0)
            # warm the PE clock (HAM) with junk matmuls while input DMAs run
            junk = small_pool.tile([128, 512], bf16, tag="junk")
            nc.vector.memset(junk[:], 0.5)
            junkw = small_pool.tile([128, 128], bf16, tag="junkw")
            nc.vector.memset(junkw[:], 0.5)
            for i in range(24):
                wps = psum_e.tile([128, 1024], f32, tag="pse", name="warm")
                nc.tensor.matmul(wps[:, 0:512], junkw[:], junk[:],
                                 start=True, stop=True)

            # ---- q/k projections (fp8 DR): qT/kT [128, S] per head pair ----
            qk_tiles = {}

            def emit_qk_proj(m, n, nm):
                wt = wq_t if nm == "q" else wk_t
                if (nm, m) not in qk_tiles:
                    qk_tiles[(nm, m)] = qk_pool.tile(
                        [128, S], bf16, tag=f"{nm}{m}", name=f"{nm}T{m}")
                dst = qk_tiles[(nm, m)]
                ps = psum_p.tile([128, 512], f32, tag="psp", name="psproj")
                for kk in range(KC):
                    k = (kk + n * 2) % KC
                    nc.tensor.matmul(
                        ps[:],
                        wc(wt, k)[:, m * 128:(m + 1) * 128],
                        xc(k)[:, n * 512:(n + 1) * 512],
                        start=(kk == 0), stop=(kk == KC - 1))
                nc.vector.tensor_copy(dst[:, n * 512:(n + 1) * 512], ps[:])

            # ---- v projection (fp8 DR): va[sc] = [128 tok, 256 ch] ----
            va = [None] * SC

            def emit_va(sc):
                t = va_pool.tile([128, CPB], bf16, tag="va", name=f"va{sc}")
                ps = psum_p.tile([128, 512], f32, tag="psp", name="psv")
                for k in range(KC):
                    nc.tensor.matmul(
                        ps[:, 0:CPB],
                        xc(k)[:, sc * 128:(sc + 1) * 128],
                        wc(wv_t, k)[:],
                        start=(k == 0), stop=(k == KC - 1))
                nc.vector.tensor_copy(t[:], ps[:, 0:CPB])
                va[sc] = t

            # ---- attention ----
            ot_t = ot_pool.tile([128, 2 * S], bf16, tag="ot", name="ot")

            def ot_c(k):
                return ot_t[:, k * S:(k + 1) * S]

            ets_map = {}

            def e_scores(m, qv, kv):
                kT_m = qk_tiles[("k", m)]
                qT_m = qk_tiles[("q", m)]
                qs = slice(qv * 256, (qv + 1) * 256)
                pss = psum_e.tile([128, 1024], f32, tag="pse", name="pss")
                for j in range(2):
                    for h in range(2):
                        kc = 2 * kv + j
                        nc.tensor.matmul(
                            pss[:, (2 * h + j) * 256:(2 * h + j + 1) * 256],
                            kT_m[64 * h:64 * (h + 1),
                                 kc * 128:(kc + 1) * 128],
                            qT_m[64 * h:64 * (h + 1), qs],
                            start=True, stop=True,
                            skip_group_check=True)
                et = exp_pool.tile([128, 1024], bf16, tag="exp", bufs=16)
                nc.scalar.activation(et[:], pss[:], EXP)
                ets_map.setdefault((m, qv), []).append((kv, et))

            def e_attn(m, qv):
                """dR + PV + normalize for one (pair, q-view)."""
                ets = ets_map[(m, qv)]
                qs = slice(qv * 256, (qv + 1) * 256)
                chunks = [(2 * kv + j, et, j)
                          for (kv, et) in ets for j in range(2)]
                nch = len(chunks)

                # denominator reduce: 4 col-groups, h0->g0/g1, h1->g2/g3.
                # Full-array zero-weight dummy deterministically sets
                # has_written on known-zero data; dR matmuls accumulate.
                dps = psum_p.tile([128, 512], f32, tag="psp", name="dps")
                nc.tensor.matmul(
                    dps[:, 0:256], zeros128[:], ets[0][1][:, 0:256],
                    start=True, stop=False, skip_group_check=True)
                order = []
                for h in range(2):
                    for i, (kc, et, j) in enumerate(chunks):
                        g = 2 * h + (i % 2)
                        order.append((g, h, i, kc, et, j))
                order = [x for pair in zip(order[:nch], order[nch:])
                         for x in pair]
                last = {}
                for (g, h, i, kc, et, j) in order:
                    last[g] = (h, i)
                for (g, h, i, kc, et, j) in order:
                    nc.tensor.matmul(
                        dps[32 * g:32 * (g + 1), 0:256],
                        ones32[:],
                        et[:, (2 * h + j) * 256:(2 * h + j + 1) * 256],
                        start=False, stop=(last[g] == (h, i)),
                        tile_position=(0, 32 * g),
                        skip_group_check=True)

                # PV: outT[ch, q] += va.T @ et, heads col-packed
                ups = psum_u.tile([128, 256], f32, tag="psu",
                                  name=f"u{m}_{qv}")
                nc.tensor.matmul(
                    ups[:], zeros128[:], ets[0][1][:, 0:256],
                    start=True, stop=False, skip_group_check=True)
                pv = []
                for i, (kc, et, j) in enumerate(chunks):
                    for h in range(2):
                        pv.append((h, i, kc, et, j))
                pv.sort(key=lambda t: (t[1], t[0]))
                for (h, i, kc, et, j) in pv:
                    nc.tensor.matmul(
                        ups[64 * h:64 * (h + 1), :],
                        va[kc][:, (2 * m + h) * 64:(2 * m + h + 1) * 64],
                        et[:, (2 * h + j) * 256:(2 * h + j + 1) * 256],
                        start=False, stop=(i == nch - 1),
                        tile_position=(0, 64 * h),
                        skip_group_check=True)

                # normalize: d = partition-sum of dps halves; R = 1/d
                ds = nrm_pool.tile([128, 256], f32, tag="ds")
                nc.vector.tensor_copy(ds[:], dps[:, 0:256])
                nc.tensor.matmul(dps[:, 256:512], map128[:], ds[:],
                                 start=True, stop=True,
                                 skip_group_check=True)
                rr = nrm_pool.tile([128, 256], f32, tag="rr")
                nc.vector.reciprocal_approx_fast(rr[:], dps[:, 256:512])
                nc.vector.tensor_tensor(
                    ot_c(m)[:, qs], ups[:], rr[:], MULT)
                del ets_map[(m, qv)]

            # ---- output projection (fp8 DR): y = ot_i.T @ wp_i ----
            def emit_outproj(sc):
                ys = ysb_pool.tile([128, C], bf16, tag="ysb")
                for n in range(2):
                    ps = psum_p.tile([128, 512], f32, tag="psp", name="psy")
                    for k in range(2):
                        nc.tensor.matmul(
                            ps[:],
                            ot_c(k)[:, sc * 128:(sc + 1) * 128],
                            wp_t[:, k * C + n * 512:k * C + n * 512 + 512],
                            start=(k == 0), stop=(k == 1))
                    nc.vector.tensor_copy(ys[:, n * 512:(n + 1) * 512],
                                          ps[:])
                nc.sync.dma_start(y[sc * 128:(sc + 1) * 128, :], ys[:])

            # ---- paced interleaved emission ----
            from collections import deque

            primary = deque()
            primary.extend([("va", sc, 1100) for sc in (2, 3, 0, 1)])
            for m, n in ((0, 1), (0, 2), (0, 3), (1, 0), (1, 1), (1, 2),
                         (1, 3)):
                primary.append(("qk", (m, n, "q"), 1800))
                primary.append(("qk", (m, n, "k"), 1800))
                if m == 0:
                    for sc in range(4 * n, 4 * n + 4):
                        primary.append(("va", sc, 1100))

            aux = deque()
            done = set()

            def run_item(it):
                kind, arg, cost = it
                if kind == "va":
                    emit_va(arg)
                elif kind == "qk":
                    emit_qk_proj(*arg)
                elif kind == "attn":
                    e_attn(*arg)
                elif kind == "op":
                    emit_outproj(arg)
                done.add((kind, arg))

            def drain(ns):
                while ns > 0 and (aux or primary):
                    it = aux.popleft() if aux else primary.popleft()
                    run_item(it)
                    ns -= it[2]

            def ensure(kind, arg):
                while (kind, arg) not in done:
                    assert primary, f"unsatisfiable ensure {kind} {arg}"
                    run_item(primary.popleft())

            emit_qk_proj(0, 0, "q")
            emit_qk_proj(0, 0, "k")
            for m in range(2):
                qv_order = range(V) if m == 0 else range(V - 1, -1, -1)
                for qv in qv_order:
                    kvs = _allowed(qv)
                    n_need = max(qv * 256 + 255,
                                 (2 * max(kvs) + 2) * 128 - 1) // 512
                    if (m, n_need) != (0, 0):
                        ensure("qk", (m, n_need, "q"))
                        ensure("qk", (m, n_need, "k"))
                    for kv in kvs:
                        e_scores(m, qv, kv)
                        drain(900)
                    for kc in range(2 * max(kvs) + 2):
                        ensure("va", kc)
                    aux.append(("attn", (m, qv), 500 + len(kvs) * 700))
                    if m == 1:
                        aux.append(("op", 2 * qv, 700))
                        aux.append(("op", 2 * qv + 1, 700))
            while aux or primary:
                run_item(aux.popleft() if aux else primary.popleft())

    nc.compile()
    return nc


def _get_compiled():
    if "nc" not in _compiled:
        _compiled["nc"] = build()
    return _compiled["nc"]


def make_in_maps(x, Wq, Wk, Wv, Wp):
    bf = ml_dtypes.bfloat16
    xf = np.asarray(x, np.float32).reshape(B, S, C)
    Wq = np.asarray(Wq, np.float32)
    Wk = np.asarray(Wk, np.float32)
    Wv = np.asarray(Wv, np.float32)
    Wp = np.asarray(Wp, np.float32)

    def pack(a, nchunk):
        # [C_, N] -> [128, nchunk*N]: row = 128*k + p
        n = a.shape[1]
        return np.ascontiguousarray(
            a.reshape(nchunk, 128, n).transpose(1, 0, 2).reshape(
                128, nchunk * n))

    in_maps = []
    for c in range(N_CORES):
        b, g = divmod(c, HPC)
        hs = slice(g * CPB, (g + 1) * CPB)
        xT = xf[b].T                                   # [C, S]
        wqT = (Wq[hs] * SCALE).T                       # [C, CPB]
        in_maps.append({
            "xi": pack(xT, 8).astype(bf),
            "wq": pack(wqT, 8).astype(bf),
            "wk": pack(Wk[hs].T, 8).astype(bf),
            "wv": pack(Wv[hs].T, 8).astype(bf),
            "wp": pack(Wp[:, hs].T, 2).astype(bf),
        })
    return in_maps


def kernel(x, Wq, Wk, Wv, Wp, bp, _trace=False, _tmpdir=None):
    global LAST_RESULTS
    from concourse import bass_utils

    nc = _get_compiled()
    in_maps = make_in_maps(x, Wq, Wk, Wv, Wp)
    kwargs = {}
    if _trace:
        kwargs = {"trace": True, "tmpdir": _tmpdir}
    res = bass_utils.run_bass_kernel_spmd(
        nc, in_maps, core_ids=list(range(N_CORES)), **kwargs)
    LAST_RESULTS = res
    yout = np.zeros((B, S, C), np.float32)
    for c in range(N_CORES):
        yout[c // HPC] += res.results[c]["y"].astype(np.float32)
    yout += np.asarray(bp, np.float32).reshape(1, 1, C)
    return yout.reshape(B, V, L, C)


# revision 27
# speedup vs baseline: 1.0354x; 1.0354x over previous
"""Block-sparse (view-causal) multi-head attention on 8 TRN2 NeuronCores.

Full inputs in, full output out. Sharding: data-parallel over batch (B=2),
tensor-parallel over heads (16 heads -> 4 per core). Each core computes its
4 heads' attention + its slice of the output projection; the host sums the
4 head-group partial projections per batch (the tensor-parallel reduce).

v2c:
- flipped PV: outT = va.T @ et with va stationary (N=256 moving free dim),
  the two heads of a pair packed into PE column groups; attention output
  lands channel-major so no PE transposes are needed before the projection.
- softmax denominators from ones-weight matmuls into the spare column
  groups, reduced+broadcast across partitions by one block-diagonal-map
  matmul, inverted with the fast DVE reciprocal.
- everything stays bf16 (fp8 measured ~5e-2 rel err: quantization noise
  in a random dot product does not average down vs the signal). Inputs
  arrive as a few large host-packed DMAs; attention scale folded into Wq.
- paced interleaved emission: the exp stream on the Scalar engine starts
  right after the first q/k projection slice; projections, PV, dR and the
  output projection drain into the gaps between score blocks. Pair 1 is
  processed in descending q-view order so the tail ends on the smallest
  views.
"""

import sys

if "/opt/trn_rl_repo" not in sys.path:
    sys.path.insert(0, "/opt/trn_rl_repo")

import numpy as np
import ml_dtypes

B, V, L, C, H = 2, 8, 256, 1024, 16
S = V * L                # 2048 tokens
DH = C // H              # 64
HPC = 4                  # heads per core
CPB = HPC * DH           # 256 channel block per core
N_CORES = 8
SCALE = DH ** -0.5       # 1/8, folded into Wq host-side
_compiled = {}
LAST_RESULTS = None


def _allowed(qv):
    """View-level mask row: views 0/1 cross-attend only; views >=2 block-causal."""
    if qv == 0:
        return [1]
    if qv == 1:
        return [0]
    return list(range(qv + 1))


def build():
    import concourse.tile as tile
    from concourse import bacc, mybir

    f32 = mybir.dt.float32
    bf16 = mybir.dt.bfloat16
    EXP = mybir.ActivationFunctionType.Exp
    MULT = mybir.AluOpType.mult

    nc = bacc.Bacc("TRN2", target_bir_lowering=False, debug=False,
                   num_devices=N_CORES)
    # host-packed layouts (row = 128*k + p for chunk k):
    #   xi [128, k(8), S] flat;  wq/wk/wv [128, k(8), CPB];  wp [128, k(2), C]
    xi_d = nc.dram_tensor("xi", [128, 8 * S], bf16, kind="ExternalInput").ap()
    wq_d = nc.dram_tensor("wq", [128, 8 * CPB], bf16,
                          kind="ExternalInput").ap()
    wk_d = nc.dram_tensor("wk", [128, 8 * CPB], bf16,
                          kind="ExternalInput").ap()
    wv_d = nc.dram_tensor("wv", [128, 8 * CPB], bf16,
                          kind="ExternalInput").ap()
    wp_d = nc.dram_tensor("wp", [128, 2 * C], bf16, kind="ExternalInput").ap()
    y = nc.dram_tensor("y", [S, C], bf16, kind="ExternalOutput").ap()

    KC = 8               # contraction chunks (of 128) for q/k/v projections
    NS = S // 512        # 4 free-dim chunks for q/k projections
    SC = S // 128        # 16 sequence chunks

    with tile.TileContext(nc) as tc:
        with (
            tc.tile_pool(name="xt", bufs=1) as xt_pool,
            tc.tile_pool(name="wts", bufs=1) as w_pool,
            tc.tile_pool(name="qk", bufs=1) as qk_pool,
            tc.tile_pool(name="va", bufs=SC) as va_pool,
            tc.tile_pool(name="ot", bufs=1) as ot_pool,
            tc.tile_pool(name="exp", bufs=3) as exp_pool,
            tc.tile_pool(name="small", bufs=1) as small_pool,
            tc.tile_pool(name="nrm", bufs=2) as nrm_pool,
            tc.tile_pool(name="ysb", bufs=3) as ysb_pool,
            tc.tile_pool(name="pse", bufs=2, space="PSUM") as psum_e,
            tc.tile_pool(name="psu", bufs=2, space="PSUM") as psum_u,
            tc.tile_pool(name="psp", bufs=2, space="PSUM") as psum_p,
        ):
            # ---- input DMAs (few, large) ----
            wq_t = w_pool.tile([128, 8 * CPB], bf16, tag="wq", name="wq")
            wk_t = w_pool.tile([128, 8 * CPB], bf16, tag="wk", name="wk")
            wv_t = w_pool.tile([128, 8 * CPB], bf16, tag="wv", name="wv")
            wp_t = w_pool.tile([128, 2 * C], bf16, tag="wp", name="wp")
            # weights first on the two fast HWDGE queues; x quarters behind
            # them; wp (needed last) on the slow gpsimd SWDGE path
            nc.sync.dma_start(wq_t[:], wq_d[:, :])
            nc.scalar.dma_start(wk_t[:], wk_d[:, :])
            nc.scalar.dma_start(wv_t[:], wv_d[:, :])
            xi_t = xt_pool.tile([128, 8 * S], bf16, tag="xi", name="xi")
            # column-quarter transfers: quarter n covers token cols
            # [512n, 512n+512) of ALL 8 chunks, so projection slice n can
            # start as soon as its quarter lands
            xiv = xi_t[:].rearrange("p (k n) -> p k n", k=8)
            xdv = xi_d.rearrange("p (k n) -> p k n", k=8)
            for i in range(4):
                q_eng = nc.sync if i % 2 == 0 else nc.scalar
                q_eng.dma_start(xiv[:, :, i * 512:(i + 1) * 512],
                                xdv[:, :, i * 512:(i + 1) * 512])
            nc.gpsimd.dma_start(wp_t[:], wp_d[:, :])

            def xc(k):
                return xi_t[:, k * S:(k + 1) * S]

            def wc(wt, k):
                return wt[:, k * CPB:(k + 1) * CPB]

            # ---- constants ----
            ones32 = small_pool.tile([128, 32], bf16, tag="ones32")
            nc.vector.memset(ones32[:], 1.0 / 32.0)
            zeros128 = small_pool.tile([128, 128], bf16, tag="zeros128")
            nc.vector.memset(zeros128[:], 0.0)
            # block-diagonal ones map: one matmul does the 64-row partition
            # reduce AND broadcasts d_h0 to rows 0-63 / d_h1 to rows 64-127
            map128 = small_pool.tile([128, 128], bf16, tag="map128")
            nc.vector.memset(map128[:], 0.0)
            nc.vector.memset(map128[0:64, 0:64], 1.0)
            nc.vector.memset(map128[64:128, 64:128], 1.0)
            # warm the PE clock (HAM) with junk matmuls while input DMAs run
            junk = small_pool.tile([128, 512], bf16, tag="junk")
            nc.vector.memset(junk[:], 0.5)
            junkw = small_pool.tile([128, 128], bf16, tag="junkw")
            nc.vector.memset(junkw[:], 0.5)
            for i in range(12):
                wps = psum_e.tile([128, 1024], f32, tag="pse", name="warm")
                nc.tensor.matmul(wps[:, 0:512], junkw[:], junk[:],
                                 start=True, stop=True)

            # ---- q/k projections (fp8 DR): qT/kT [128, S] per head pair ----
            qk_tiles = {}

            def emit_qk_proj(m, n, nm):
                wt = wq_t if nm == "q" else wk_t
                if (nm, m) not in qk_tiles:
                    qk_tiles[(nm, m)] = qk_pool.tile(
                        [128, S], bf16, tag=f"{nm}{m}", name=f"{nm}T{m}")
                dst = qk_tiles[(nm, m)]
                ps = psum_p.tile([128, 512], f32, tag="psp", name="psproj")
                for kk in range(KC):
                    k = (kk + n * 2) % KC
                    nc.tensor.matmul(
                        ps[:],
                        wc(wt, k)[:, m * 128:(m + 1) * 128],
                        xc(k)[:, n * 512:(n + 1) * 512],
                        start=(kk == 0), stop=(kk == KC - 1))
                nc.vector.tensor_copy(dst[:, n * 512:(n + 1) * 512], ps[:])

            # ---- v projection (fp8 DR): va[sc] = [128 tok, 256 ch] ----
            va = [None] * SC

            def emit_va(sc):
                t = va_pool.tile([128, CPB], bf16, tag="va", name=f"va{sc}")
                ps = psum_p.tile([128, 512], f32, tag="psp", name="psv")
                for k in range(KC):
                    nc.tensor.matmul(
                        ps[:, 0:CPB],
                        xc(k)[:, sc * 128:(sc + 1) * 128],
                        wc(wv_t, k)[:],
                        start=(k == 0), stop=(k == KC - 1))
                nc.vector.tensor_copy(t[:], ps[:, 0:CPB])
                va[sc] = t

            # ---- attention ----
            ot_t = ot_pool.tile([128, 2 * S], bf16, tag="ot", name="ot")

            def ot_c(k):
                return ot_t[:, k * S:(k + 1) * S]

            ets_map = {}

            def e_scores(m, qv, kv):
                kT_m = qk_tiles[("k", m)]
                qT_m = qk_tiles[("q", m)]
                qs = slice(qv * 256, (qv + 1) * 256)
                pss = psum_e.tile([128, 1024], f32, tag="pse", name="pss")
                for j in range(2):
                    for h in range(2):
                        kc = 2 * kv + j
                        nc.tensor.matmul(
                            pss[:, (2 * h + j) * 256:(2 * h + j + 1) * 256],
                            kT_m[64 * h:64 * (h + 1),
                                 kc * 128:(kc + 1) * 128],
                            qT_m[64 * h:64 * (h + 1), qs],
                            start=True, stop=True,
                            skip_group_check=True)
                et = exp_pool.tile([128, 1024], bf16, tag="exp", bufs=16)
                nc.scalar.activation(et[:], pss[:], EXP)
                ets_map.setdefault((m, qv), []).append((kv, et))

            def e_attn(m, qv):
                """dR + PV + normalize for one (pair, q-view)."""
                ets = ets_map[(m, qv)]
                qs = slice(qv * 256, (qv + 1) * 256)
                chunks = [(2 * kv + j, et, j)
                          for (kv, et) in ets for j in range(2)]
                nch = len(chunks)

                # denominator reduce: 4 col-groups, h0->g0/g1, h1->g2/g3.
                # Full-array zero-weight dummy deterministically sets
                # has_written on known-zero data; dR matmuls accumulate.
                dps = psum_p.tile([128, 512], f32, tag="psp", name="dps")
                nc.tensor.matmul(
                    dps[:, 0:256], zeros128[:], ets[0][1][:, 0:256],
                    start=True, stop=False, skip_group_check=True)
                order = []
                for h in range(2):
                    for i, (kc, et, j) in enumerate(chunks):
                        g = 2 * h + (i % 2)
                        order.append((g, h, i, kc, et, j))
                order = [x for pair in zip(order[:nch], order[nch:])
                         for x in pair]
                last = {}
                for (g, h, i, kc, et, j) in order:
                    last[g] = (h, i)
                for (g, h, i, kc, et, j) in order:
                    nc.tensor.matmul(
                        dps[32 * g:32 * (g + 1), 0:256],
                        ones32[:],
                        et[:, (2 * h + j) * 256:(2 * h + j + 1) * 256],
                        start=False, stop=(last[g] == (h, i)),
                        tile_position=(0, 32 * g),
                        skip_group_check=True)

                # PV: outT[ch, q] += va.T @ et, heads col-packed
                ups = psum_u.tile([128, 256], f32, tag="psu",
                                  name=f"u{m}_{qv}")
                nc.tensor.matmul(
                    ups[:], zeros128[:], ets[0][1][:, 0:256],
                    start=True, stop=False, skip_group_check=True)
                pv = []
                for i, (kc, et, j) in enumerate(chunks):
                    for h in range(2):
                        pv.append((h, i, kc, et, j))
                pv.sort(key=lambda t: (t[1], t[0]))
                for (h, i, kc, et, j) in pv:
                    nc.tensor.matmul(
                        ups[64 * h:64 * (h + 1), :],
                        va[kc][:, (2 * m + h) * 64:(2 * m + h + 1) * 64],
                        et[:, (2 * h + j) * 256:(2 * h + j + 1) * 256],
                        start=False, stop=(i == nch - 1),
                        tile_position=(0, 64 * h),
                        skip_group_check=True)

                # normalize: d = partition-sum of dps halves; R = 1/d
                ds = nrm_pool.tile([128, 256], bf16, tag="ds")
                nc.vector.tensor_copy(ds[:], dps[:, 0:256])
                nc.tensor.matmul(dps[:, 256:512], map128[:], ds[:],
                                 start=True, stop=True,
                                 skip_group_check=True)
                rr = nrm_pool.tile([128, 256], f32, tag="rr")
                nc.vector.reciprocal_approx_fast(rr[:], dps[:, 256:512])
                nc.vector.tensor_tensor(
                    ot_c(m)[:, qs], ups[:], rr[:], MULT)
                del ets_map[(m, qv)]

            # ---- output projection (fp8 DR): y = ot_i.T @ wp_i ----
            def emit_outproj(sc):
                ys = ysb_pool.tile([128, C], bf16, tag="ysb")
                for n in range(2):
                    ps = psum_p.tile([128, 512], f32, tag="psp", name="psy")
                    for k in range(2):
                        nc.tensor.matmul(
                            ps[:],
                            ot_c(k)[:, sc * 128:(sc + 1) * 128],
                            wp_t[:, k * C + n * 512:k * C + n * 512 + 512],
                            start=(k == 0), stop=(k == 1))
                    if n == 0:
                        nc.vector.tensor_copy(ys[:, 0:512], ps[:])
                    else:
                        nc.scalar.copy(ys[:, 512:1024], ps[:])
                nc.sync.dma_start(y[sc * 128:(sc + 1) * 128, :], ys[:])

            # ---- paced interleaved emission ----
            from collections import deque

            primary = deque()
            primary.extend([("va", sc, 1100) for sc in (2, 3, 0, 1)])
            for m, n in ((0, 1), (0, 2), (0, 3), (1, 0), (1, 1), (1, 2),
                         (1, 3)):
                primary.append(("qk", (m, n, "q"), 1800))
                primary.append(("qk", (m, n, "k"), 1800))
                if m == 0:
                    for sc in range(4 * n, 4 * n + 4):
                        primary.append(("va", sc, 1100))

            aux = deque()
            done = set()

            def run_item(it):
                kind, arg, cost = it
                if kind == "va":
                    emit_va(arg)
                elif kind == "qk":
                    emit_qk_proj(*arg)
                elif kind == "attn":
                    e_attn(*arg)
                elif kind == "op":
                    emit_outproj(arg)
                done.add((kind, arg))

            def drain(ns):
                while ns > 0 and (aux or primary):
                    it = aux.popleft() if aux else primary.popleft()
                    run_item(it)
                    ns -= it[2]

            def ensure(kind, arg):
                while (kind, arg) not in done:
                    assert primary, f"unsatisfiable ensure {kind} {arg}"
                    run_item(primary.popleft())

            emit_qk_proj(0, 0, "q")
            emit_qk_proj(0, 0, "k")
            for m in range(2):
                qv_order = range(V) if m == 0 else range(V - 1, -1, -1)
                for qv in qv_order:
                    kvs = _allowed(qv)
                    n_need = max(qv * 256 + 255,
                                 (2 * max(kvs) + 2) * 128 - 1) // 512
                    if (m, n_need) != (0, 0):
                        ensure("qk", (m, n_need, "q"))
                        ensure("qk", (m, n_need, "k"))
                    for kv in kvs:
                        e_scores(m, qv, kv)
                        drain(900 if m == 0 else 1300)
                    for kc in range(2 * max(kvs) + 2):
                        ensure("va", kc)
                    aux.append(("attn", (m, qv), 500 + len(kvs) * 700))
                    if m == 1:
                        aux.append(("op", 2 * qv, 700))
                        aux.append(("op", 2 * qv + 1, 700))
            while aux or primary:
                run_item(aux.popleft() if aux else primary.popleft())

    nc.compile()
    return nc


def _get_compiled():
    if "nc" not in _compiled:
        _compiled["nc"] = build()
    return _compiled["nc"]


def make_in_maps(x, Wq, Wk, Wv, Wp):
    bf = ml_dtypes.bfloat16
    xf = np.asarray(x, np.float32).reshape(B, S, C)
    Wq = np.asarray(Wq, np.float32)
    Wk = np.asarray(Wk, np.float32)
    Wv = np.asarray(Wv, np.float32)
    Wp = np.asarray(Wp, np.float32)

    def pack(a, nchunk):
        # [C_, N] -> [128, nchunk*N]: row = 128*k + p
        n = a.shape[1]
        return np.ascontiguousarray(
            a.reshape(nchunk, 128, n).transpose(1, 0, 2).reshape(
                128, nchunk * n))

    in_maps = []
    for c in range(N_CORES):
        b, g = divmod(c, HPC)
        hs = slice(g * CPB, (g + 1) * CPB)
        xT = xf[b].T                                   # [C, S]
        wqT = (Wq[hs] * SCALE).T                       # [C, CPB]
        in_maps.append({
            "xi": pack(xT, 8).astype(bf),
            "wq": pack(wqT, 8).astype(bf),
            "wk": pack(Wk[hs].T, 8).astype(bf),
            "wv": pack(Wv[hs].T, 8).astype(bf),
            "wp": pack(Wp[:, hs].T, 2).astype(bf),
        })
    return in_maps


def kernel(x, Wq, Wk, Wv, Wp, bp, _trace=False, _tmpdir=None):
    global LAST_RESULTS
    from concourse import bass_utils

    nc = _get_compiled()
    in_maps = make_in_maps(x, Wq, Wk, Wv, Wp)
    kwargs = {}
    if _trace:
        kwargs = {"trace": True, "tmpdir": _tmpdir}
    res = bass_utils.run_bass_kernel_spmd(
        nc, in_maps, core_ids=list(range(N_CORES)), **kwargs)
    LAST_RESULTS = res
    yout = np.zeros((B, S, C), np.float32)
    for c in range(N_CORES):
        yout[c // HPC] += res.results[c]["y"].astype(np.float32)
    yout += np.asarray(bp, np.float32).reshape(1, 1, C)
    return yout.reshape(B, V, L, C)


# revision 29
# speedup vs baseline: 1.0636x; 1.0273x over previous
"""Block-sparse (view-causal) multi-head attention on 8 TRN2 NeuronCores.

Full inputs in, full output out. Sharding: data-parallel over batch (B=2),
tensor-parallel over heads (16 heads -> 4 per core). Each core computes its
4 heads' attention + its slice of the output projection; the host sums the
4 head-group partial projections per batch (the tensor-parallel reduce).

v2c:
- flipped PV: outT = va.T @ et with va stationary (N=256 moving free dim),
  the two heads of a pair packed into PE column groups; attention output
  lands channel-major so no PE transposes are needed before the projection.
- softmax denominators from ones-weight matmuls into the spare column
  groups, reduced+broadcast across partitions by one block-diagonal-map
  matmul, inverted with the fast DVE reciprocal.
- everything stays bf16 (fp8 measured ~5e-2 rel err: quantization noise
  in a random dot product does not average down vs the signal). Inputs
  arrive as a few large host-packed DMAs; attention scale folded into Wq.
- paced interleaved emission: the exp stream on the Scalar engine starts
  right after the first q/k projection slice; projections, PV, dR and the
  output projection drain into the gaps between score blocks. Pair 1 is
  processed in descending q-view order so the tail ends on the smallest
  views.
"""

import sys

if "/opt/trn_rl_repo" not in sys.path:
    sys.path.insert(0, "/opt/trn_rl_repo")

import numpy as np
import ml_dtypes

B, V, L, C, H = 2, 8, 256, 1024, 16
S = V * L                # 2048 tokens
DH = C // H              # 64
HPC = 4                  # heads per core
CPB = HPC * DH           # 256 channel block per core
N_CORES = 8
SCALE = DH ** -0.5       # 1/8, folded into Wq host-side
_compiled = {}
LAST_RESULTS = None


def _allowed(qv):
    """View-level mask row: views 0/1 cross-attend only; views >=2 block-causal."""
    if qv == 0:
        return [1]
    if qv == 1:
        return [0]
    return list(range(qv + 1))


def build():
    import concourse.tile as tile
    from concourse import bacc, mybir

    f32 = mybir.dt.float32
    bf16 = mybir.dt.bfloat16
    EXP = mybir.ActivationFunctionType.Exp
    MULT = mybir.AluOpType.mult

    nc = bacc.Bacc("TRN2", target_bir_lowering=False, debug=False,
                   num_devices=N_CORES)
    # host-packed layouts (row = 128*k + p for chunk k):
    #   xi [128, nq(4), k(8), 512] flat (quarter-major so each token-column
    #   quarter is one contiguous DMA); wq/wk/wv [128, k(8), CPB];
    #   wp [128, k(2), C]
    xi_d = nc.dram_tensor("xi", [128, 8 * S], bf16, kind="ExternalInput").ap()
    wq_d = nc.dram_tensor("wq", [128, 8 * CPB], bf16,
                          kind="ExternalInput").ap()
    wk_d = nc.dram_tensor("wk", [128, 8 * CPB], bf16,
                          kind="ExternalInput").ap()
    wv_d = nc.dram_tensor("wv", [128, 8 * CPB], bf16,
                          kind="ExternalInput").ap()
    wp_d = nc.dram_tensor("wp", [128, 2 * C], bf16, kind="ExternalInput").ap()
    y = nc.dram_tensor("y", [S, C], bf16, kind="ExternalOutput").ap()

    KC = 8               # contraction chunks (of 128) for q/k/v projections
    NS = S // 512        # 4 free-dim chunks for q/k projections
    SC = S // 128        # 16 sequence chunks

    with tile.TileContext(nc) as tc:
        with (
            tc.tile_pool(name="xt", bufs=1) as xt_pool,
            tc.tile_pool(name="wts", bufs=1) as w_pool,
            tc.tile_pool(name="qk", bufs=1) as qk_pool,
            tc.tile_pool(name="va", bufs=SC) as va_pool,
            tc.tile_pool(name="ot", bufs=1) as ot_pool,
            tc.tile_pool(name="exp", bufs=3) as exp_pool,
            tc.tile_pool(name="small", bufs=1) as small_pool,
            tc.tile_pool(name="nrm", bufs=2) as nrm_pool,
            tc.tile_pool(name="ysb", bufs=3) as ysb_pool,
            tc.tile_pool(name="pse", bufs=2, space="PSUM") as psum_e,
            tc.tile_pool(name="psu", bufs=2, space="PSUM") as psum_u,
            tc.tile_pool(name="psp", bufs=2, space="PSUM") as psum_p,
        ):
            # ---- input DMAs (few, large) ----
            wq_t = w_pool.tile([128, 8 * CPB], bf16, tag="wq", name="wq")
            wk_t = w_pool.tile([128, 8 * CPB], bf16, tag="wk", name="wk")
            wv_t = w_pool.tile([128, 8 * CPB], bf16, tag="wv", name="wv")
            wp_t = w_pool.tile([128, 2 * C], bf16, tag="wp", name="wp")
            # two fast HWDGE queues; x token-quarters are contiguous in
            # the host layout. wp (needed last) rides the slow SWDGE path.
            xi_t = xt_pool.tile([128, 8 * S], bf16, tag="xi", name="xi")
            QB = 8 * 512
            nc.sync.dma_start(wv_t[:], wv_d[:, :])
            nc.scalar.dma_start(wq_t[:], wq_d[:, :])
            nc.scalar.dma_start(wk_t[:], wk_d[:, :])
            nc.sync.dma_start(xi_t[:, 0:QB], xi_d[:, 0:QB])
            nc.scalar.dma_start(xi_t[:, QB:2 * QB], xi_d[:, QB:2 * QB])
            nc.sync.dma_start(xi_t[:, 2 * QB:3 * QB], xi_d[:, 2 * QB:3 * QB])
            nc.scalar.dma_start(xi_t[:, 3 * QB:4 * QB],
                                xi_d[:, 3 * QB:4 * QB])
            nc.gpsimd.dma_start(wp_t[:], wp_d[:, :])

            def xq_slice(k, lo, hi):
                # token columns [lo, hi) of chunk k (within one quarter)
                nq = lo // 512
                base = nq * 8 * 512 + k * 512
                return xi_t[:, base + lo % 512:base + (hi - 1) % 512 + 1]

            def wc(wt, k):
                return wt[:, k * CPB:(k + 1) * CPB]

            # ---- constants ----
            ones32 = small_pool.tile([128, 32], bf16, tag="ones32")
            nc.vector.memset(ones32[:], 1.0 / 32.0)
            zeros128 = small_pool.tile([128, 128], bf16, tag="zeros128")
            nc.vector.memset(zeros128[:], 0.0)
            # block-diagonal ones map: one matmul does the 64-row partition
            # reduce AND broadcasts d_h0 to rows 0-63 / d_h1 to rows 64-127
            map128 = small_pool.tile([128, 128], bf16, tag="map128")
            nc.vector.memset(map128[:], 0.0)
            nc.vector.memset(map128[0:64, 0:64], 1.0)
            nc.vector.memset(map128[64:128, 64:128], 1.0)
            # warm the PE clock (HAM) with junk matmuls while input DMAs run
            junk = small_pool.tile([128, 512], bf16, tag="junk")
            nc.vector.memset(junk[:], 0.5)
            junkw = small_pool.tile([128, 128], bf16, tag="junkw")
            nc.vector.memset(junkw[:], 0.5)
            for i in range(12):
                wps = psum_e.tile([128, 1024], f32, tag="pse", name="warm")
                nc.tensor.matmul(wps[:, 0:512], junkw[:], junk[:],
                                 start=True, stop=True)

            # ---- q/k projections (fp8 DR): qT/kT [128, S] per head pair ----
            qk_tiles = {}

            def emit_qk_proj(m, n, nm):
                wt = wq_t if nm == "q" else wk_t
                if (nm, m) not in qk_tiles:
                    qk_tiles[(nm, m)] = qk_pool.tile(
                        [128, S], bf16, tag=f"{nm}{m}", name=f"{nm}T{m}")
                dst = qk_tiles[(nm, m)]
                ps = psum_p.tile([128, 512], f32, tag="psp", name="psproj")
                for kk in range(KC):
                    k = (kk + n * 2) % KC
                    nc.tensor.matmul(
                        ps[:],
                        wc(wt, k)[:, m * 128:(m + 1) * 128],
                        xq_slice(k, n * 512, (n + 1) * 512),
                        start=(kk == 0), stop=(kk == KC - 1))
                nc.vector.tensor_copy(dst[:, n * 512:(n + 1) * 512], ps[:])

            # ---- v projection (fp8 DR): va[sc] = [128 tok, 256 ch] ----
            va = [None] * SC

            def emit_va(sc):
                t = va_pool.tile([128, CPB], bf16, tag="va", name=f"va{sc}")
                ps = psum_p.tile([128, 512], f32, tag="psp", name="psv")
                for k in range(KC):
                    nc.tensor.matmul(
                        ps[:, 0:CPB],
                        xq_slice(k, sc * 128, (sc + 1) * 128),
                        wc(wv_t, k)[:],
                        start=(k == 0), stop=(k == KC - 1))
                nc.vector.tensor_copy(t[:], ps[:, 0:CPB])
                va[sc] = t

            # ---- attention ----
            ot_t = ot_pool.tile([128, 2 * S], bf16, tag="ot", name="ot")

            def ot_c(k):
                return ot_t[:, k * S:(k + 1) * S]

            ets_map = {}

            def e_scores(m, qv, kv):
                kT_m = qk_tiles[("k", m)]
                qT_m = qk_tiles[("q", m)]
                qs = slice(qv * 256, (qv + 1) * 256)
                pss = psum_e.tile([128, 1024], f32, tag="pse", name="pss")
                for j in range(2):
                    for h in range(2):
                        kc = 2 * kv + j
                        nc.tensor.matmul(
                            pss[:, (2 * h + j) * 256:(2 * h + j + 1) * 256],
                            kT_m[64 * h:64 * (h + 1),
                                 kc * 128:(kc + 1) * 128],
                            qT_m[64 * h:64 * (h + 1), qs],
                            start=True, stop=True,
                            skip_group_check=True)
                et = exp_pool.tile([128, 1024], bf16, tag="exp", bufs=16)
                nc.scalar.activation(et[:], pss[:], EXP)
                ets_map.setdefault((m, qv), []).append((kv, et))

            def e_attn(m, qv):
                """dR + PV + normalize for one (pair, q-view)."""
                ets = ets_map[(m, qv)]
                qs = slice(qv * 256, (qv + 1) * 256)
                chunks = [(2 * kv + j, et, j)
                          for (kv, et) in ets for j in range(2)]
                nch = len(chunks)

                # denominator reduce: 4 col-groups, h0->g0/g1, h1->g2/g3.
                # Full-array zero-weight dummy deterministically sets
                # has_written on known-zero data; dR matmuls accumulate.
                dps = psum_p.tile([128, 512], f32, tag="psp", name="dps")
                nc.tensor.matmul(
                    dps[:, 0:256], zeros128[:], ets[0][1][:, 0:256],
                    start=True, stop=False, skip_group_check=True)
                order = []
                for h in range(2):
                    for i, (kc, et, j) in enumerate(chunks):
                        g = 2 * h + (i % 2)
                        order.append((g, h, i, kc, et, j))
                order = [x for pair in zip(order[:nch], order[nch:])
                         for x in pair]
                last = {}
                for (g, h, i, kc, et, j) in order:
                    last[g] = (h, i)
                for (g, h, i, kc, et, j) in order:
                    nc.tensor.matmul(
                        dps[32 * g:32 * (g + 1), 0:256],
                        ones32[:],
                        et[:, (2 * h + j) * 256:(2 * h + j + 1) * 256],
                        start=False, stop=(last[g] == (h, i)),
                        tile_position=(0, 32 * g),
                        skip_group_check=True)

                # PV: outT[ch, q] += va.T @ et, heads col-packed
                ups = psum_u.tile([128, 256], f32, tag="psu",
                                  name=f"u{m}_{qv}")
                nc.tensor.matmul(
                    ups[:], zeros128[:], ets[0][1][:, 0:256],
                    start=True, stop=False, skip_group_check=True)
                pv = []
                for i, (kc, et, j) in enumerate(chunks):
                    for h in range(2):
                        pv.append((h, i, kc, et, j))
                pv.sort(key=lambda t: (t[1], t[0]))
                for (h, i, kc, et, j) in pv:
                    nc.tensor.matmul(
                        ups[64 * h:64 * (h + 1), :],
                        va[kc][:, (2 * m + h) * 64:(2 * m + h + 1) * 64],
                        et[:, (2 * h + j) * 256:(2 * h + j + 1) * 256],
                        start=False, stop=(i == nch - 1),
                        tile_position=(0, 64 * h),
                        skip_group_check=True)

                # normalize: d = partition-sum of dps halves; R = 1/d
                ds = nrm_pool.tile([128, 256], bf16, tag="ds")
                nc.vector.tensor_copy(ds[:], dps[:, 0:256])
                nc.tensor.matmul(dps[:, 256:512], map128[:], ds[:],
                                 start=True, stop=True,
                                 skip_group_check=True)
                rr = nrm_pool.tile([128, 256], f32, tag="rr")
                nc.vector.reciprocal_approx_fast(rr[:], dps[:, 256:512])
                nc.vector.tensor_tensor(
                    ot_c(m)[:, qs], ups[:], rr[:], MULT)
                del ets_map[(m, qv)]

            # ---- output projection (fp8 DR): y = ot_i.T @ wp_i ----
            def emit_outproj(sc):
                ys = ysb_pool.tile([128, C], bf16, tag="ysb")
                for n in range(2):
                    ps = psum_p.tile([128, 512], f32, tag="psp", name="psy")
                    for k in range(2):
                        nc.tensor.matmul(
                            ps[:],
                            ot_c(k)[:, sc * 128:(sc + 1) * 128],
                            wp_t[:, k * C + n * 512:k * C + n * 512 + 512],
                            start=(k == 0), stop=(k == 1))
                    if n == 0:
                        nc.vector.tensor_copy(ys[:, 0:512], ps[:])
                    else:
                        nc.scalar.copy(ys[:, 512:1024], ps[:])
                nc.sync.dma_start(y[sc * 128:(sc + 1) * 128, :], ys[:])

            # ---- paced interleaved emission ----
            from collections import deque

            primary = deque()
            primary.extend([("va", sc, 1100) for sc in (2, 3, 0, 1)])
            for m, n in ((0, 1), (0, 2), (0, 3), (1, 0), (1, 1), (1, 2),
                         (1, 3)):
                primary.append(("qk", (m, n, "q"), 1800))
                primary.append(("qk", (m, n, "k"), 1800))
                if m == 0:
                    for sc in range(4 * n, 4 * n + 4):
                        primary.append(("va", sc, 1100))

            aux = deque()
            reserve = []
            done = set()

            def run_item(it):
                kind, arg, cost = it
                if kind == "va":
                    emit_va(arg)
                elif kind == "qk":
                    emit_qk_proj(*arg)
                elif kind == "attn":
                    e_attn(*arg)
                elif kind == "op":
                    emit_outproj(arg)
                done.add((kind, arg))

            def drain(ns):
                while ns > 0 and (aux or primary):
                    it = aux.popleft() if aux else primary.popleft()
                    run_item(it)
                    ns -= it[2]

            def ensure(kind, arg):
                while (kind, arg) not in done:
                    assert primary, f"unsatisfiable ensure {kind} {arg}"
                    run_item(primary.popleft())

            emit_qk_proj(0, 0, "q")
            emit_qk_proj(0, 0, "k")
            for m in range(2):
                qv_order = range(V) if m == 0 else range(V - 1, -1, -1)
                for qv in qv_order:
                    kvs = _allowed(qv)
                    n_need = max(qv * 256 + 255,
                                 (2 * max(kvs) + 2) * 128 - 1) // 512
                    if (m, n_need) != (0, 0):
                        ensure("qk", (m, n_need, "q"))
                        ensure("qk", (m, n_need, "k"))
                    for kv in kvs:
                        e_scores(m, qv, kv)
                        drain(900 if m == 0 else 1300)
                    for kc in range(2 * max(kvs) + 2):
                        ensure("va", kc)
                    aux.append(("attn", (m, qv), 500 + len(kvs) * 700))
                    if m == 1:
                        if qv in (2, 3):
                            reserve.append(("op", 2 * qv, 700))
                            reserve.append(("op", 2 * qv + 1, 700))
                        else:
                            aux.append(("op", 2 * qv, 700))
                            aux.append(("op", 2 * qv + 1, 700))
            # reserved outproj tiles fill the PE while the last exps finish
            for it in reserve:
                run_item(it)
            while aux or primary:
                run_item(aux.popleft() if aux else primary.popleft())

    nc.compile()
    return nc


def _get_compiled():
    if "nc" not in _compiled:
        _compiled["nc"] = build()
    return _compiled["nc"]


def make_in_maps(x, Wq, Wk, Wv, Wp):
    bf = ml_dtypes.bfloat16
    xf = np.asarray(x, np.float32).reshape(B, S, C)
    Wq = np.asarray(Wq, np.float32)
    Wk = np.asarray(Wk, np.float32)
    Wv = np.asarray(Wv, np.float32)
    Wp = np.asarray(Wp, np.float32)

    def pack(a, nchunk):
        # [C_, N] -> [128, nchunk*N]: row = 128*k + p
        n = a.shape[1]
        return np.ascontiguousarray(
            a.reshape(nchunk, 128, n).transpose(1, 0, 2).reshape(
                128, nchunk * n))

    in_maps = []
    for c in range(N_CORES):
        b, g = divmod(c, HPC)
        hs = slice(g * CPB, (g + 1) * CPB)
        xT = xf[b].T                                   # [C, S]
        wqT = (Wq[hs] * SCALE).T                       # [C, CPB]
        xq = pack(xT, 8).reshape(128, 8, 4, 512).transpose(
            0, 2, 1, 3).reshape(128, 8 * S)            # quarter-major
        in_maps.append({
            "xi": np.ascontiguousarray(xq).astype(bf),
            "wq": pack(wqT, 8).astype(bf),
            "wk": pack(Wk[hs].T, 8).astype(bf),
            "wv": pack(Wv[hs].T, 8).astype(bf),
            "wp": pack(Wp[:, hs].T, 2).astype(bf),
        })
    return in_maps


def kernel(x, Wq, Wk, Wv, Wp, bp, _trace=False, _tmpdir=None):
    global LAST_RESULTS
    from concourse import bass_utils

    nc = _get_compiled()
    in_maps = make_in_maps(x, Wq, Wk, Wv, Wp)
    kwargs = {}
    if _trace:
        kwargs = {"trace": True, "tmpdir": _tmpdir}
    res = bass_utils.run_bass_kernel_spmd(
        nc, in_maps, core_ids=list(range(N_CORES)), **kwargs)
    LAST_RESULTS = res
    yout = np.zeros((B, S, C), np.float32)
    for c in range(N_CORES):
        yout[c // HPC] += res.results[c]["y"].astype(np.float32)
    yout += np.asarray(bp, np.float32).reshape(1, 1, C)
    return yout.reshape(B, V, L, C)
